# revision 9
# baseline (speedup 1.0000x reference)
"""BetaTCVAE loss kernel for 8 Trainium2 NeuronCores.

Math: reference computes
    kl_loss = sum(kl)
    log_qz_prob[i,j,l] = -0.5*((z_i_l - m_j_l)^2 * exp(-v_j_l) + v_j_l + LOG2PI)
    log_qz_product[i]  = sum_l logsumexp_j log_qz_prob[i,j,l]
    log_qz[i]          = logsumexp_j sum_l log_qz_prob[i,j,l]
    out = (BETA-1)*mean_i(log_qz - log_qz_product) + kl_loss

Key transform: with w = exp(-v),
    log_qz_prob[i,j,l] = a[j,l]*z2[i,l] + b[j,l]*z[i,l] + g[j,l]
      a = -w/2, b = w*m, g = -(w*m^2 + v + LOG2PI)/2, z2 = z^2
so the [i,j] exp-argument per l is a K=3 matmul (TensorE), and the full
sum over l (for log_qz) is a K=3L matmul. The only O(B^2*L) elementwise
pass is a single ScalarE Exp with fused free-dim accumulation (accum_out),
which reduces over j in the same instruction.

Sharding: outer batch dim i split across 8 cores (256 rows each); the
[B,L] coefficient tensors are replicated. Per-core partial sums are
combined on host (the trivial all-reduce).
"""

import os
import sys
from contextlib import ExitStack

import numpy as np

for _p in ("/opt/trn_rl_repo", "/root/.axon_site/_ro/trn_rl_repo"):
    if os.path.isdir(_p) and _p not in sys.path:
        sys.path.append(_p)

import concourse.bass as bass
import concourse.tile as tile
from concourse import mybir

BETA = 6.0
LOG_2PI = float(np.log(2.0 * np.pi))
F32 = mybir.dt.float32
BF16 = mybir.dt.bfloat16
AF = mybir.ActivationFunctionType


def build_nc(B=2048, L=64, BC=256, split_waits=True):
    """Build the per-core Bass program.

    B: total batch (j dim, replicated on every core)
    L: latent dim
    BC: rows of i handled by this core
    """
    PI = 128
    assert BC % PI == 0
    nit = BC // PI
    JT = min(512, B)
    assert B % JT == 0
    njc = B // JT
    KS = 3 * L
    KC = 96 if KS % 96 == 0 else KS
    assert KS % KC == 0
    nkc = KS // KC
    scale_r = (BETA - 1.0) / float(B)

    nc = bass.Bass()
    zpk_d = nc.declare_dram_parameter("zpk", [nit, 3, L * PI], BF16, False)
    zs_d = nc.declare_dram_parameter("zs", [nkc, KC, BC], F32, False)
    coefd_d = nc.declare_dram_parameter("coefd", [L, 3, B], BF16, False)
    coefs_d = nc.declare_dram_parameter("coefs", [nkc, KC, B], F32, False)
    kld_d = nc.declare_dram_parameter("kld", [BC, L], F32, False)
    out_d = nc.declare_dram_parameter("out", [1, 1], F32, True)

    with tile.TileContext(nc) as tc, ExitStack() as ctx:
        const_pool = ctx.enter_context(tc.tile_pool(name="const", bufs=1))
        work = ctx.enter_context(tc.tile_pool(name="work", bufs=2))
        coefl_pool = ctx.enter_context(tc.tile_pool(name="coefl", bufs=4))
        small = ctx.enter_context(tc.tile_pool(name="small", bufs=1))
        psum = ctx.enter_context(tc.tile_pool(name="psum", bufs=2, space="PSUM"))

        # --- persistent loads ---
        zpk_t = []
        for it in range(nit):
            t = const_pool.tile([128, L * PI], BF16, tag=f"zpk{it}", name=f"zpk{it}")
            for g in range(4):
                nc.sync.dma_start(out=t[32 * g:32 * g + 3, :], in_=zpk_d[it])
            zpk_t.append(t)
        zs_t = []
        coefs_t = []
        for k in range(nkc):
            t = const_pool.tile([KC, BC], F32, tag=f"zs{k}", name=f"zs{k}")
            nc.sync.dma_start(out=t[:], in_=zs_d[k])
            zs_t.append(t)
            t2 = const_pool.tile([KC, B], F32, tag=f"cs{k}", name=f"cs{k}")
            nc.sync.dma_start(out=t2[:], in_=coefs_d[k])
            coefs_t.append(t2)
        kl_t = []
        for it in range(nit):
            t = const_pool.tile([PI, L], F32, tag=f"kl{it}", name=f"klt{it}")
            nc.sync.dma_start(out=t[:], in_=kld_d[it * PI:(it + 1) * PI, :])
            kl_t.append(t)
        ones_t = small.tile([PI, 1], F32, tag="ones")
        nc.any.memset(ones_t[:], 1.0)

        # --- phase A: log_qz via S = sum_l arg, logsumexp over j ---
        lq_t = []
        for it in range(nit):
            sp = psum.tile([PI, B], F32, tag="bigpsum")
            for k in range(nkc):
                lhsT = zs_t[k][:, it * PI:(it + 1) * PI]
                for jc in range(njc):
                    nc.tensor.matmul(
                        sp[:, jc * JT:(jc + 1) * JT],
                        lhsT,
                        coefs_t[k][:, jc * JT:(jc + 1) * JT],
                        start=(k == 0),
                        stop=(k == nkc - 1),
                    )
            mx = small.tile([PI, 1], F32, tag=f"mx{it}")
            nc.vector.tensor_reduce(mx[:], sp[:], axis=mybir.AxisListType.X,
                                    op=mybir.AluOpType.max)
            negmx = small.tile([PI, 1], F32, tag=f"negmx{it}")
            nc.scalar.mul(negmx[:], mx[:], -1.0)
            es = work.tile([PI, B], F32, tag="es")
            sume = small.tile([PI, 1], F32, tag=f"sume{it}")
            nc.scalar.activation(es[:], sp[:], AF.Exp, bias=negmx[:], scale=1.0,
                                 accum_out=sume[:])
            lq = small.tile([PI, 1], F32, tag=f"lq{it}")
            nc.scalar.activation(lq[:], sume[:], AF.Ln)
            nc.vector.tensor_add(lq[:], lq[:], mx[:])
            lq_t.append(lq)

        # --- phase B: log_qz_product; G[i,l] = sum_j exp(arg_l[i,j]) ---
        g_t = [small.tile([PI, L], F32, tag=f"g{it}", name=f"g{it}") for it in range(nit)]
        for l in range(L):
            cf = coefl_pool.tile([128, B], BF16, tag="cf")
            for g in range(4):
                nc.sync.dma_start(out=cf[32 * g:32 * g + 3, :], in_=coefd_d[l])
            for it in range(nit):
                ap = psum.tile([PI, B], F32, tag="bigpsum")
                g = (l * nit + it) % 4
                lhsT = zpk_t[it][32 * g:32 * g + 3, l * PI:(l + 1) * PI]
                for jc in range(njc):
                    nc.tensor.matmul(
                        ap[:, jc * JT:(jc + 1) * JT],
                        lhsT,
                        cf[32 * g:32 * g + 3, jc * JT:(jc + 1) * JT],
                        start=True,
                        stop=True,
                        tile_position=(32 * g, 0),
                    )
                ed = work.tile([PI, B], F32, tag="ed")
                nc.scalar.activation(ed[:], ap[:], AF.Exp,
                                     accum_out=g_t[it][:, l:l + 1])

        # --- combine per-core: r = (lq - sum_l ln G) * (BETA-1)/B ; kl sums ---
        fin = psum.tile([1, 1], F32, tag="bigpsum")
        vecs = []
        for it in range(nit):
            logg = small.tile([PI, L], F32, tag=f"logg{it}")
            nc.scalar.activation(logg[:], g_t[it][:], AF.Ln)
            lqp = small.tile([PI, 1], F32, tag=f"lqp{it}")
            nc.vector.tensor_reduce(lqp[:], logg[:], axis=mybir.AxisListType.X,
                                    op=mybir.AluOpType.add)
            r = small.tile([PI, 1], F32, tag=f"r{it}")
            nc.vector.tensor_sub(r[:], lq_t[it][:], lqp[:])
            nc.scalar.mul(r[:], r[:], scale_r)
            vecs.append(r)
            kls = small.tile([PI, 1], F32, tag=f"kls{it}")
            nc.vector.tensor_reduce(kls[:], kl_t[it][:], axis=mybir.AxisListType.X,
                                    op=mybir.AluOpType.add)
            vecs.append(kls)
        for vi, v in enumerate(vecs):
            nc.tensor.matmul(fin[:], v[:], ones_t[:],
                             start=(vi == 0), stop=(vi == len(vecs) - 1))
        ob = small.tile([1, 1], F32, tag="ob")
        nc.scalar.copy(ob[:], fin[:])
        nc.sync.dma_start(out=out_d[:], in_=ob[:])

    return _split_multi_waits(nc) if split_waits else nc


def _split_multi_waits(nc):
    """Walrus (gen3 codegen) accepts at most ONE sync-wait per instruction.
    Tile's wait assignment can attach several. Split the extras onto NoOp
    instructions on the same engine immediately before the instruction —
    same-engine streams execute in order, so semantics are preserved."""
    wid = [0]

    def fix_block(b):
        new = []
        for inst in b.instructions:
            si = inst.sync_info
            if si is not None and si.on_wait and len(si.on_wait) > 1:
                for w in si.on_wait[:-1]:
                    wid[0] += 1
                    nop = mybir.InstNoOp(
                        name=f"WSPLIT-{wid[0]}",
                        engine=inst.engine,
                        sync_info=mybir.SyncInfo(on_wait=[w], on_update=[]),
                    )
                    nop.bass_nofuse = True
                    new.append(nop)
                si.on_wait = [si.on_wait[-1]]
            new.append(inst)
        b.instructions[:] = new

    for fn in nc.m.functions:
        for b in fn.blocks:
            fix_block(b)
    return nc


def make_inputs(kl, z_mean, z_logvar, z_sampled, n_cores):
    """Host-side O(B*L) prep: coefficient tensors + per-core shards."""
    B, L = kl.shape
    BC = B // n_cores
    PI = 128
    nit = BC // PI
    KS = 3 * L
    KC = 96 if KS % 96 == 0 else KS
    nkc = KS // KC

    kl = np.ascontiguousarray(kl, dtype=np.float32)
    m = np.asarray(z_mean, dtype=np.float32)
    v = np.asarray(z_logvar, dtype=np.float32)
    z = np.asarray(z_sampled, dtype=np.float32)

    w = np.exp(-v)
    a = -0.5 * w
    b = w * m
    g = -0.5 * (w * m * m + v + LOG_2PI)
    import ml_dtypes
    coefd = np.ascontiguousarray(
        np.stack([a, b, g], 0).transpose(2, 0, 1)).astype(ml_dtypes.bfloat16)  # [L, 3, B]
    coefs = np.ascontiguousarray(
        np.stack([a, b, g], 0).transpose(2, 0, 1).reshape(3 * L, B)
        .reshape(nkc, KC, B))                             # [nkc, KC, B] f32

    in_maps = []
    for c in range(n_cores):
        zc = z[c * BC:(c + 1) * BC]                       # [BC, L]
        arr = np.stack([zc * zc, zc, np.ones_like(zc)], 0)  # [3, BC, L]
        zs = np.ascontiguousarray(
            arr.transpose(2, 0, 1).reshape(3 * L, BC).reshape(nkc, KC, BC))
        arrT = arr.transpose(0, 2, 1)                     # [3, L, BC]
        zpk = np.stack(
            [arrT[:, :, it * PI:(it + 1) * PI].reshape(3, L * PI)
             for it in range(nit)], 0).astype(ml_dtypes.bfloat16)  # [nit, 3, L*PI]
        in_maps.append({
            "zpk": np.ascontiguousarray(zpk),
            "zs": zs,
            "coefd": coefd,
            "coefs": coefs,
            "kld": np.ascontiguousarray(kl[c * BC:(c + 1) * BC]),
        })
    return in_maps


_NC_CACHE = {}


def _get_nc(B, L, BC):
    key = (B, L, BC)
    if key not in _NC_CACHE:
        _NC_CACHE[key] = build_nc(B, L, BC)
    return _NC_CACHE[key]


def _enable_jax_cache():
    try:
        import jax
        jax.config.update("jax_compilation_cache_dir", "/tmp/jaxcache")
        jax.config.update("jax_persistent_cache_min_entry_size_bytes", 0)
        jax.config.update("jax_persistent_cache_min_compile_time_secs", 0)
    except Exception:
        pass


def kernel(kl, z_mean, z_logvar, z_sampled):
    from concourse.bass_utils import run_bass_kernel_spmd

    _enable_jax_cache()

    B, L = kl.shape
    n_cores = 8
    BC = B // n_cores
    nc = _get_nc(B, L, BC)
    in_maps = make_inputs(kl, z_mean, z_logvar, z_sampled, n_cores)
    res = run_bass_kernel_spmd(nc, in_maps, list(range(n_cores)))
    total = sum(float(r["out"][0, 0]) for r in res.results)
    return np.float32(total)


# revision 10
# speedup vs baseline: 1.0117x; 1.0117x over previous
"""BetaTCVAE loss kernel for 8 Trainium2 NeuronCores.

Math: reference computes
    kl_loss = sum(kl)
    log_qz_prob[i,j,l] = -0.5*((z_i_l - m_j_l)^2 * exp(-v_j_l) + v_j_l + LOG2PI)
    log_qz_product[i]  = sum_l logsumexp_j log_qz_prob[i,j,l]
    log_qz[i]          = logsumexp_j sum_l log_qz_prob[i,j,l]
    out = (BETA-1)*mean_i(log_qz - log_qz_product) + kl_loss

Key transform: with w = exp(-v),
    log_qz_prob[i,j,l] = a[j,l]*z2[i,l] + b[j,l]*z[i,l] + g[j,l]
      a = -w/2, b = w*m, g = -(w*m^2 + v + LOG2PI)/2, z2 = z^2
so the [i,j] exp-argument per l is a K=3 matmul (TensorE), and the full
sum over l (for log_qz) is a K=3L matmul. The only O(B^2*L) elementwise
pass is a single ScalarE Exp with fused free-dim accumulation (accum_out),
which reduces over j in the same instruction.

Sharding: outer batch dim i split across 8 cores (256 rows each); the
[B,L] coefficient tensors are replicated. Per-core partial sums are
combined on host (the trivial all-reduce).
"""

import os
import sys
from contextlib import ExitStack

import numpy as np

for _p in ("/opt/trn_rl_repo", "/root/.axon_site/_ro/trn_rl_repo"):
    if os.path.isdir(_p) and _p not in sys.path:
        sys.path.append(_p)

import concourse.bass as bass
import concourse.tile as tile
from concourse import mybir

BETA = 6.0
LOG_2PI = float(np.log(2.0 * np.pi))
F32 = mybir.dt.float32
BF16 = mybir.dt.bfloat16
AF = mybir.ActivationFunctionType


def build_nc(B=2048, L=64, BC=256, split_waits=True):
    """Build the per-core Bass program.

    B: total batch (j dim, replicated on every core)
    L: latent dim
    BC: rows of i handled by this core
    """
    PI = 128
    assert BC % PI == 0
    nit = BC // PI
    JT = min(512, B)
    assert B % JT == 0
    njc = B // JT
    KS = 3 * L
    KC = 96 if KS % 96 == 0 else KS
    assert KS % KC == 0
    nkc = KS // KC
    scale_r = (BETA - 1.0) / float(B)

    nc = bass.Bass()
    zpk_d = nc.declare_dram_parameter("zpk", [nit, 3, L * PI], BF16, False)
    zs_d = nc.declare_dram_parameter("zs", [nkc, KC, BC], F32, False)
    coefd_d = nc.declare_dram_parameter("coefd", [L, 3, B], BF16, False)
    coefs_d = nc.declare_dram_parameter("coefs", [nkc, KC, B], F32, False)
    kld_d = nc.declare_dram_parameter("kld", [BC, L], F32, False)
    out_d = nc.declare_dram_parameter("out", [1, 1], F32, True)

    with tile.TileContext(nc) as tc, ExitStack() as ctx:
        const_pool = ctx.enter_context(tc.tile_pool(name="const", bufs=1))
        work = ctx.enter_context(tc.tile_pool(name="work", bufs=2))
        coefl_pool = ctx.enter_context(tc.tile_pool(name="coefl", bufs=4))
        small = ctx.enter_context(tc.tile_pool(name="small", bufs=1))
        psum = ctx.enter_context(tc.tile_pool(name="psum", bufs=2, space="PSUM"))

        # --- persistent loads ---
        zpk_t = []
        for it in range(nit):
            t = const_pool.tile([128, L * PI], BF16, tag=f"zpk{it}", name=f"zpk{it}")
            for g in range(4):
                nc.sync.dma_start(out=t[32 * g:32 * g + 3, :], in_=zpk_d[it])
            zpk_t.append(t)
        # --- phase B: log_qz_product; G[i,l] = sum_j exp(arg_l[i,j]) ---
        g_t = [small.tile([PI, L], F32, tag=f"g{it}", name=f"g{it}") for it in range(nit)]
        for l in range(L):
            cf = coefl_pool.tile([128, B], BF16, tag="cf")
            for g in range(4):
                nc.sync.dma_start(out=cf[32 * g:32 * g + 3, :], in_=coefd_d[l])
            for it in range(nit):
                ap = psum.tile([PI, B], F32, tag="bigpsum")
                g = (l * nit + it) % 4
                lhsT = zpk_t[it][32 * g:32 * g + 3, l * PI:(l + 1) * PI]
                for jc in range(njc):
                    nc.tensor.matmul(
                        ap[:, jc * JT:(jc + 1) * JT],
                        lhsT,
                        cf[32 * g:32 * g + 3, jc * JT:(jc + 1) * JT],
                        start=True,
                        stop=True,
                        tile_position=(32 * g, 0),
                    )
                ed = work.tile([PI, B], F32, tag="ed")
                nc.scalar.activation(ed[:], ap[:], AF.Exp,
                                     accum_out=g_t[it][:, l:l + 1])

        # --- loads for phase A / kl (issued late: off the critical head) ---
        zs_t = []
        coefs_t = []
        for k in range(nkc):
            t = const_pool.tile([KC, BC], F32, tag=f"zs{k}", name=f"zs{k}")
            nc.sync.dma_start(out=t[:], in_=zs_d[k])
            zs_t.append(t)
            t2 = const_pool.tile([KC, B], F32, tag=f"cs{k}", name=f"cs{k}")
            nc.sync.dma_start(out=t2[:], in_=coefs_d[k])
            coefs_t.append(t2)
        kl_t = []
        for it in range(nit):
            t = const_pool.tile([PI, L], F32, tag=f"kl{it}", name=f"klt{it}")
            nc.sync.dma_start(out=t[:], in_=kld_d[it * PI:(it + 1) * PI, :])
            kl_t.append(t)
        ones_t = small.tile([PI, 1], F32, tag="ones")
        nc.any.memset(ones_t[:], 1.0)

        # --- phase A: log_qz via S = sum_l arg, logsumexp over j ---
        lq_t = []
        for it in range(nit):
            sp = psum.tile([PI, B], F32, tag="bigpsum")
            for k in range(nkc):
                lhsT = zs_t[k][:, it * PI:(it + 1) * PI]
                for jc in range(njc):
                    nc.tensor.matmul(
                        sp[:, jc * JT:(jc + 1) * JT],
                        lhsT,
                        coefs_t[k][:, jc * JT:(jc + 1) * JT],
                        start=(k == 0),
                        stop=(k == nkc - 1),
                    )
            mx = small.tile([PI, 1], F32, tag=f"mx{it}")
            nc.vector.tensor_reduce(mx[:], sp[:], axis=mybir.AxisListType.X,
                                    op=mybir.AluOpType.max)
            negmx = small.tile([PI, 1], F32, tag=f"negmx{it}")
            nc.scalar.mul(negmx[:], mx[:], -1.0)
            es = work.tile([PI, B], F32, tag="es")
            sume = small.tile([PI, 1], F32, tag=f"sume{it}")
            nc.scalar.activation(es[:], sp[:], AF.Exp, bias=negmx[:], scale=1.0,
                                 accum_out=sume[:])
            lq = small.tile([PI, 1], F32, tag=f"lq{it}")
            nc.scalar.activation(lq[:], sume[:], AF.Ln)
            nc.vector.tensor_add(lq[:], lq[:], mx[:])
            lq_t.append(lq)

        # --- combine per-core: r = (lq - sum_l ln G) * (BETA-1)/B ; kl sums ---
        fin = psum.tile([1, 1], F32, tag="bigpsum")
        vecs = []
        for it in range(nit):
            logg = small.tile([PI, L], F32, tag=f"logg{it}")
            nc.scalar.activation(logg[:], g_t[it][:], AF.Ln)
            lqp = small.tile([PI, 1], F32, tag=f"lqp{it}")
            nc.vector.tensor_reduce(lqp[:], logg[:], axis=mybir.AxisListType.X,
                                    op=mybir.AluOpType.add)
            r = small.tile([PI, 1], F32, tag=f"r{it}")
            nc.vector.tensor_sub(r[:], lq_t[it][:], lqp[:])
            nc.scalar.mul(r[:], r[:], scale_r)
            vecs.append(r)
            kls = small.tile([PI, 1], F32, tag=f"kls{it}")
            nc.vector.tensor_reduce(kls[:], kl_t[it][:], axis=mybir.AxisListType.X,
                                    op=mybir.AluOpType.add)
            vecs.append(kls)
        for vi, v in enumerate(vecs):
            nc.tensor.matmul(fin[:], v[:], ones_t[:],
                             start=(vi == 0), stop=(vi == len(vecs) - 1))
        ob = small.tile([1, 1], F32, tag="ob")
        nc.scalar.copy(ob[:], fin[:])
        nc.sync.dma_start(out=out_d[:], in_=ob[:])

    return _split_multi_waits(nc) if split_waits else nc


def _split_multi_waits(nc):
    """Walrus (gen3 codegen) accepts at most ONE sync-wait per instruction.
    Tile's wait assignment can attach several. Split the extras onto NoOp
    instructions on the same engine immediately before the instruction —
    same-engine streams execute in order, so semantics are preserved."""
    wid = [0]

    def fix_block(b):
        new = []
        for inst in b.instructions:
            si = inst.sync_info
            if si is not None and si.on_wait and len(si.on_wait) > 1:
                for w in si.on_wait[:-1]:
                    wid[0] += 1
                    nop = mybir.InstNoOp(
                        name=f"WSPLIT-{wid[0]}",
                        engine=inst.engine,
                        sync_info=mybir.SyncInfo(on_wait=[w], on_update=[]),
                    )
                    nop.bass_nofuse = True
                    new.append(nop)
                si.on_wait = [si.on_wait[-1]]
            new.append(inst)
        b.instructions[:] = new

    for fn in nc.m.functions:
        for b in fn.blocks:
            fix_block(b)
    return nc


def make_inputs(kl, z_mean, z_logvar, z_sampled, n_cores):
    """Host-side O(B*L) prep: coefficient tensors + per-core shards."""
    B, L = kl.shape
    BC = B // n_cores
    PI = 128
    nit = BC // PI
    KS = 3 * L
    KC = 96 if KS % 96 == 0 else KS
    nkc = KS // KC

    kl = np.ascontiguousarray(kl, dtype=np.float32)
    m = np.asarray(z_mean, dtype=np.float32)
    v = np.asarray(z_logvar, dtype=np.float32)
    z = np.asarray(z_sampled, dtype=np.float32)

    w = np.exp(-v)
    a = -0.5 * w
    b = w * m
    g = -0.5 * (w * m * m + v + LOG_2PI)
    import ml_dtypes
    coefd = np.ascontiguousarray(
        np.stack([a, b, g], 0).transpose(2, 0, 1)).astype(ml_dtypes.bfloat16)  # [L, 3, B]
    coefs = np.ascontiguousarray(
        np.stack([a, b, g], 0).transpose(2, 0, 1).reshape(3 * L, B)
        .reshape(nkc, KC, B))                             # [nkc, KC, B] f32

    in_maps = []
    for c in range(n_cores):
        zc = z[c * BC:(c + 1) * BC]                       # [BC, L]
        arr = np.stack([zc * zc, zc, np.ones_like(zc)], 0)  # [3, BC, L]
        zs = np.ascontiguousarray(
            arr.transpose(2, 0, 1).reshape(3 * L, BC).reshape(nkc, KC, BC))
        arrT = arr.transpose(0, 2, 1)                     # [3, L, BC]
        zpk = np.stack(
            [arrT[:, :, it * PI:(it + 1) * PI].reshape(3, L * PI)
             for it in range(nit)], 0).astype(ml_dtypes.bfloat16)  # [nit, 3, L*PI]
        in_maps.append({
            "zpk": np.ascontiguousarray(zpk),
            "zs": zs,
            "coefd": coefd,
            "coefs": coefs,
            "kld": np.ascontiguousarray(kl[c * BC:(c + 1) * BC]),
        })
    return in_maps


_NC_CACHE = {}


def _get_nc(B, L, BC):
    key = (B, L, BC)
    if key not in _NC_CACHE:
        _NC_CACHE[key] = build_nc(B, L, BC)
    return _NC_CACHE[key]


def _enable_jax_cache():
    try:
        import jax
        jax.config.update("jax_compilation_cache_dir", "/tmp/jaxcache")
        jax.config.update("jax_persistent_cache_min_entry_size_bytes", 0)
        jax.config.update("jax_persistent_cache_min_compile_time_secs", 0)
    except Exception:
        pass


def kernel(kl, z_mean, z_logvar, z_sampled):
    from concourse.bass_utils import run_bass_kernel_spmd

    _enable_jax_cache()

    B, L = kl.shape
    n_cores = 8
    BC = B // n_cores
    nc = _get_nc(B, L, BC)
    in_maps = make_inputs(kl, z_mean, z_logvar, z_sampled, n_cores)
    res = run_bass_kernel_spmd(nc, in_maps, list(range(n_cores)))
    total = sum(float(r["out"][0, 0]) for r in res.results)
    return np.float32(total)
